# revision 23
# baseline (speedup 1.0000x reference)
"""Trainium2 Bass kernel for nn_CPCModule (CPC loss: GRU encoder + NCE contrast).

Self-contained: hardcodes shapes/sharding. Stage 1 shards the batch (512) over
8 NeuronCores for the GRU scan; stage 2 shards the F=16 contrast heads over the
8 cores. z_e_x / z_q_x are pure passthrough outputs and never touch the device.
"""
import os
from contextlib import ExitStack

import numpy as np

import concourse.bass as bass
import concourse.mybir as mybir
import concourse.tile as tile
from concourse import bacc
from concourse.bass_utils import run_bass_kernel_spmd

LAST_EXEC_NS = 0


def _run(nc, in_maps):
    global LAST_EXEC_NS
    import time
    trace = os.environ.get("KERNEL_TRACE") == "1"
    t0 = time.perf_counter_ns()
    r = run_bass_kernel_spmd(nc, in_maps, list(range(NC)), trace=trace)
    dt = time.perf_counter_ns() - t0
    if r.exec_time_ns:
        LAST_EXEC_NS += int(r.exec_time_ns)
    else:
        LAST_EXEC_NS += dt
    return r.results

F32 = mybir.dt.float32
AF = mybir.ActivationFunctionType
ALU = mybir.AluOpType

NC = 8
B, K, HH, WW = 512, 256, 16, 16
KH = 128
F = 16
HW = HH * WW
BL = B // NC  # 64 batch rows per core (stage 1)
FL = F // NC  # 2 heads per core (stage 2)

_CACHE = {}


# --------------------------------------------------------------------------- #
# stage 1: GRU over the causal prefix; emits c_t^T (KH, BL) per core
# --------------------------------------------------------------------------- #
def build_stage1(T, has_bias):
    nc = bacc.Bacc()
    zs = nc.declare_dram_parameter("zs", [BL, K, T], F32, isOutput=False)
    wih = nc.declare_dram_parameter("wih", [K, 3 * KH], F32, isOutput=False)  # W_ih.T
    whh = nc.declare_dram_parameter("whh", [KH, 3 * KH], F32, isOutput=False)  # W_hh.T
    h0 = nc.declare_dram_parameter("h0", [KH, BL], F32, isOutput=False)
    ident = nc.declare_dram_parameter("ident", [128, 128], F32, isOutput=False)
    if has_bias:
        # rows: 0 = b_ih+b_hh (rz, cols 0:256), 1 = b_ih n-part (cols 0:128),
        # 2 = b_hh n-part (cols 0:128)
        biases = nc.declare_dram_parameter("biases", [1, 3, 2 * KH], F32, isOutput=False)
    ct = nc.declare_dram_parameter("ct", [KH, BL], F32, isOutput=True)

    WIN = 8  # timesteps per psum window tile; [128, WIN, 4*BL] = 4 banks
    n_win = (T + WIN - 1) // WIN

    with tile.TileContext(nc) as tc, ExitStack() as ctx:
        consts = ctx.enter_context(tc.tile_pool(name="consts", bufs=1))
        seqp = ctx.enter_context(tc.tile_pool(name="seq", bufs=1))
        hpool = ctx.enter_context(tc.tile_pool(name="h", bufs=3))
        scratch = ctx.enter_context(tc.tile_pool(name="scr", bufs=4))
        psum = ctx.enter_context(tc.tile_pool(name="ps", bufs=2, space="PSUM"))

        # ---- load constants ----
        wih_t = consts.tile([128, 2, 3 * KH], F32, tag="wih")  # [k-part, kc, g]
        nc.sync.dma_start(wih_t[:], wih[:].rearrange("(kc p) g -> p kc g", p=128))
        whh_t = consts.tile([128, 3 * KH], F32, tag="whh")
        nc.sync.dma_start(whh_t[:], whh[:])
        id_t = consts.tile([128, 128], F32, tag="id")
        nc.sync.dma_start(id_t[:], ident[:])
        if has_bias:
            bias_t = consts.tile([1, 3, 2 * KH], F32, tag="bias")
            nc.sync.dma_start(bias_t[:], biases[:])
            ones_t = consts.tile([1, BL], F32, tag="ones")
            nc.vector.memset(ones_t[:], 1.0)

        # ---- seqT tiles [k-part, kc, b, t]; DMA split over b for parallelism
        seq_t = seqp.tile([128, 2, BL, T], F32, tag="seq")
        for kc in range(2):
            for bq in range(2):
                bs = bq * (BL // 2)
                be = bs + BL // 2
                nc.sync.dma_start(
                    seq_t[:, kc, bs:be, :],
                    zs[bs:be, kc * 128:(kc + 1) * 128, :].rearrange("b k t -> k b t"),
                )

        # ---- initial hidden ----
        h_t = hpool.tile([KH, BL], F32, tag="h")
        nc.sync.dma_start(h_t[:], h0[:])

        # column layout within a step slot [128, 4*BL]:
        #   0:BL      r   (x part + h part)
        #   BL:2BL    z
        #   2BL:3BL   xn (+ t1 via identity-matmul)
        #   3BL:4BL   hn (h part only)
        def blk(wt, tb, b0, b1):
            return wt[:, tb, b0 * BL:b1 * BL]

        for w in range(n_win):
            t0 = w * WIN
            tn = min(WIN, T - t0)
            wt = psum.tile([128, WIN, 4 * BL], F32, tag="win")

            # x-side prefill: weights stationary per (gate chunk, k chunk),
            # one matmul per timestep in the window (single-bank psum writes).
            # start=True only on the first matmul touching each bank: start
            # marks the WHOLE 2KB bank pending-zero, so each region's first
            # writer gets fresh-write semantics and later writers accumulate.
            for g in range(3):
                for kc in range(2):
                    for tt in range(tn):
                        nc.tensor.matmul(
                            blk(wt, tt, g, g + 1),
                            wih_t[:, kc, g * 128:(g + 1) * 128],
                            seq_t[:, kc, :, t0 + tt],
                            start=(g == 0 and kc == 0 and tt % 2 == 0),
                            stop=False,
                            skip_group_check=True,
                        )

            for tt in range(tn):
                t = t0 + tt
                if has_bias:
                    for g in range(3):
                        dst = 3 if g == 2 else g
                        row, col = (2, 0) if g == 2 else (0, g * 128)
                        nc.tensor.matmul(
                            blk(wt, tt, dst, dst + 1),
                            bias_t[:, row, col:col + 128],
                            ones_t[:],
                            start=False,
                            stop=False,
                            skip_group_check=True,
                        )
                    nc.tensor.matmul(
                        blk(wt, tt, 2, 3),
                        bias_t[:, 1, 0:128],
                        ones_t[:],
                        start=False, stop=False, skip_group_check=True,
                    )
                # h-side matmuls
                for g in range(3):
                    dst = 3 if g == 2 else g  # n-gate h-part goes to hn col
                    nc.tensor.matmul(
                        blk(wt, tt, dst, dst + 1),
                        whh_t[:, g * 128:(g + 1) * 128],
                        h_t[:],
                        start=False,
                        stop=True,
                        skip_group_check=True,
                    )

                # rz = sigmoid(psum rz cols)
                rz = scratch.tile([128, 2 * BL], F32, tag="rz")
                nc.scalar.activation(rz[:], blk(wt, tt, 0, 2), AF.Sigmoid)
                # t1 = r * hn
                t1 = scratch.tile([128, BL], F32, tag="t1")
                nc.vector.tensor_tensor(t1[:], rz[:, 0:BL], blk(wt, tt, 3, 4), ALU.mult)
                # xn += t1 (identity matmul accumulate)
                nc.tensor.matmul(
                    blk(wt, tt, 2, 3), id_t[:], t1[:],
                    start=False, stop=True, skip_group_check=True,
                )
                # n = tanh(xn + t1)
                n_t = scratch.tile([128, BL], F32, tag="n")
                nc.scalar.activation(n_t[:], blk(wt, tt, 2, 3), AF.Tanh)
                # h' = n + z*(h - n)
                d_t = scratch.tile([128, BL], F32, tag="d")
                nc.vector.tensor_tensor(d_t[:], h_t[:], n_t[:], ALU.subtract)
                v_t = scratch.tile([128, BL], F32, tag="v")
                nc.vector.tensor_tensor(v_t[:], rz[:, BL:2 * BL], d_t[:], ALU.mult)
                h_t = hpool.tile([KH, BL], F32, tag="h")
                nc.vector.tensor_tensor(h_t[:], n_t[:], v_t[:], ALU.add)

        nc.sync.dma_start(ct[:], h_t[:])
    if not nc.is_finalized():
        nc.finalize()
    return nc


# --------------------------------------------------------------------------- #
# stage 2: per-core FL heads; totals, log-softmax diag, nce partials, last-f
# --------------------------------------------------------------------------- #
def build_stage2():
    nc = bacc.Bacc()
    cT = nc.declare_dram_parameter("cT", [KH, B], F32, isOutput=False)
    encT = nc.declare_dram_parameter("encT", [FL, K, B], F32, isOutput=False)
    wkT = nc.declare_dram_parameter("wkT", [FL, KH, K], F32, isOutput=False)
    wkb = nc.declare_dram_parameter("wkb", [FL, K], F32, isOutput=False)
    masks = nc.declare_dram_parameter("masks", [4, 128, B], F32, isOutput=False)
    nce_acc = nc.declare_dram_parameter("nce_acc", [128, 1], F32, isOutput=True)
    tlast = nc.declare_dram_parameter("tlast", [4, 128, B], F32, isOutput=True)
    lse_out = nc.declare_dram_parameter("lse_out", [4, 128], F32, isOutput=True)

    with tile.TileContext(nc) as tc, ExitStack() as ctx:
        consts = ctx.enter_context(tc.tile_pool(name="consts", bufs=1))
        sb = ctx.enter_context(tc.tile_pool(name="sb", bufs=3))
        small = ctx.enter_context(tc.tile_pool(name="small", bufs=8))
        psum = ctx.enter_context(tc.tile_pool(name="ps", bufs=2, space="PSUM"))
        psum_t = ctx.enter_context(tc.tile_pool(name="pst", bufs=2, space="PSUM"))

        cT_t = consts.tile([KH, B], F32, tag="cT")
        nc.sync.dma_start(cT_t[:], cT[:])
        mask_t = consts.tile([128, 4, B], F32, tag="masks")
        nc.sync.dma_start(mask_t[:], masks[:].rearrange("c p b -> p c b"))
        wkb_t = consts.tile([128, FL, 2], F32, tag="wkb")
        nc.sync.dma_start(wkb_t[:], wkb[:].rearrange("f (kc p) -> p f kc", p=128))
        wk_t = consts.tile([128, FL, K], F32, tag="wk")
        nc.sync.dma_start(wk_t[:], wkT[:].rearrange("f p k -> p f k"))
        enc_t = consts.tile([128, FL, 2, B], F32, tag="enc")
        nc.sync.dma_start(enc_t[:], encT[:].rearrange("f (kc p) b -> p f kc b", p=128))

        acc = small.tile([128, 1], F32, tag="acc")
        nc.vector.memset(acc[:], 0.0)

        for fi in range(FL):
            # predT [k-chunk part, B] per k chunk
            predT = sb.tile([128, 2, B], F32, tag="pred")
            for kc in range(2):
                pp = psum.tile([128, B], F32, tag="pp")
                nc.tensor.matmul(
                    pp[:], wk_t[:, fi, kc * 128:(kc + 1) * 128], cT_t[:],
                    start=True, stop=True,
                )
                nc.scalar.activation(
                    predT[:, kc, :], pp[:], AF.Identity,
                    bias=wkb_t[:, fi, kc:kc + 1],
                )

            for cb in range(4):
                tp = psum_t.tile([128, B], F32, tag="tp")
                for kc in range(2):
                    nc.tensor.matmul(
                        tp[:], enc_t[:, fi, kc, cb * 128:(cb + 1) * 128],
                        predT[:, kc, :],
                        start=(kc == 0), stop=(kc == 1),
                    )
                negmax = small.tile([128, 1], F32, tag="negmax")
                nc.vector.tensor_reduce(
                    negmax[:], tp[:], mybir.AxisListType.X, ALU.max, negate=True,
                )
                escr = sb.tile([128, B], F32, tag="escr")
                rowsum = small.tile([128, 1], F32, tag="rowsum")
                nc.scalar.activation(
                    escr[:], tp[:], AF.Exp, bias=negmax[:], accum_out=rowsum[:],
                )
                dscr = sb.tile([128, B], F32, tag="dscr")
                diag = small.tile([128, 1], F32, tag="diag")
                nc.vector.tensor_tensor(dscr[:], tp[:], mask_t[:, cb, :], ALU.mult)
                nc.vector.tensor_reduce(
                    diag[:], dscr[:], mybir.AxisListType.X, ALU.add)
                lnz = small.tile([128, 1], F32, tag="lnz")
                nc.scalar.activation(lnz[:], rowsum[:], AF.Ln)
                # lse = lnz - negmax ; contrib = diag - lse ; acc += contrib
                lse = small.tile([128, 1], F32, tag="lse")
                nc.vector.tensor_tensor(lse[:], lnz[:], negmax[:], ALU.subtract)
                contrib = small.tile([128, 1], F32, tag="contrib")
                nc.vector.tensor_tensor(contrib[:], diag[:], lse[:], ALU.subtract)
                acc2 = small.tile([128, 1], F32, tag="acc")
                nc.vector.tensor_tensor(acc2[:], acc[:], contrib[:], ALU.add)
                acc = acc2

                if fi == FL - 1:
                    tcp = sb.tile([128, B], F32, tag="tcp")
                    nc.scalar.activation(tcp[:], tp[:], AF.Copy)
                    nc.sync.dma_start(tlast[cb], tcp[:])
                    nc.sync.dma_start(lse_out[cb:cb + 1].rearrange("o p -> p o"), lse[:])

        nc.sync.dma_start(nce_acc[:], acc[:])
    if not nc.is_finalized():
        nc.finalize()
    return nc


# --------------------------------------------------------------------------- #
# host orchestration
# --------------------------------------------------------------------------- #
def kernel(z_q_x_st, z_e_x, z_q_x, hidden, W_ih, W_hh, b_ih, b_hh, Wk_w, Wk_b,
           p_sample):
    global LAST_EXEC_NS
    LAST_EXEC_NS = 0
    z_q_x_st = np.asarray(z_q_x_st, dtype=np.float32)
    hidden_np = np.asarray(hidden, dtype=np.float32)
    W_ih = np.asarray(W_ih, dtype=np.float32)
    W_hh = np.asarray(W_hh, dtype=np.float32)
    b_ih = np.asarray(b_ih, dtype=np.float32)
    b_hh = np.asarray(b_hh, dtype=np.float32)
    Wk_w = np.asarray(Wk_w, dtype=np.float32)
    Wk_b = np.asarray(Wk_b, dtype=np.float32)
    ps = int(p_sample)
    T = ps + 1

    has_bias = bool(np.any(b_ih) or np.any(b_hh))

    key1 = ("s1", T, has_bias)
    if key1 not in _CACHE:
        _CACHE[key1] = build_stage1(T, has_bias)
    nc1 = _CACHE[key1]

    zs = z_q_x_st.reshape(B, K, HW)
    wihT = np.ascontiguousarray(W_ih.T)
    whhT = np.ascontiguousarray(W_hh.T)
    ident = np.eye(128, dtype=np.float32)
    if has_bias:
        biases = np.zeros((1, 3, 2 * KH), np.float32)
        biases[0, 0] = (b_ih[:2 * KH] + b_hh[:2 * KH])
        biases[0, 1, :KH] = b_ih[2 * KH:]
        biases[0, 2, :KH] = b_hh[2 * KH:]

    in_maps1 = []
    for i in range(NC):
        m = {
            "zs": np.ascontiguousarray(zs[i * BL:(i + 1) * BL, :, :T]),
            "wih": wihT,
            "whh": whhT,
            "h0": np.ascontiguousarray(hidden_np[0, i * BL:(i + 1) * BL].T),
            "ident": ident,
        }
        if has_bias:
            m["biases"] = biases
        in_maps1.append(m)

    res1 = _run(nc1, in_maps1)
    c_tT = np.ascontiguousarray(
        np.concatenate([r["ct"] for r in res1], axis=1))  # (KH, B)

    key2 = ("s2",)
    if key2 not in _CACHE:
        _CACHE[key2] = build_stage2()
    nc2 = _CACHE[key2]

    encT = np.ascontiguousarray(
        zs[:, :, ps + 1:ps + 1 + F].transpose(2, 1, 0))  # (F, K, B)
    wkT = np.ascontiguousarray(Wk_w.transpose(0, 2, 1))  # (F, KH, K)
    masks = np.zeros((4, 128, B), np.float32)
    ii = np.arange(128)
    for cb in range(4):
        masks[cb, ii, cb * 128 + ii] = 1.0

    in_maps2 = []
    for i in range(NC):
        in_maps2.append({
            "cT": c_tT,
            "encT": np.ascontiguousarray(encT[i * FL:(i + 1) * FL]),
            "wkT": np.ascontiguousarray(wkT[i * FL:(i + 1) * FL]),
            "wkb": np.ascontiguousarray(Wk_b[i * FL:(i + 1) * FL]),
            "masks": masks,
        })

    res2 = _run(nc2, in_maps2)

    nce_total = sum(float(r["nce_acc"].sum()) for r in res2)
    nce = np.float32(-nce_total / (B * F))

    t15 = res2[NC - 1]["tlast"].reshape(B, B)
    lse15 = res2[NC - 1]["lse_out"].reshape(B, 1)
    sm = np.exp(t15 - lse15)
    preds = sm.argmax(axis=0)
    correct = int((preds == np.arange(B)).sum())
    accuracy = np.float32(correct / B)

    return accuracy, nce, np.asarray(z_e_x), np.asarray(z_q_x)
